# revision 3
# baseline (speedup 1.0000x reference)
"""LUT-based 3x3 conv (CustomAnyConv2d) as a Trainium2 Bass kernel.

Math: out[b,co,y,x] = bias[co] +
        sum_{ci,kh,kw} lut[ input_pad[b,ci,y+kh,x+kw], weight[co,ci,kh,kw] ]
(zero-padding pads with CODE 0, which is a valid LUT row -- matches reference).

Strategy (8 NeuronCores, data-parallel over batch, B=8 -> 1 image/core):
  For each input channel ci we build a one-hot plane over the 256 code values
  of the zero-padded 58x58 image:  oh[a, p] = (code[p] == a), fp16, stored as
  two 128-partition tiles (codes 0..127 / 128..255).  For each of the 9 taps
  (kh,kw) the contribution  sum_a oh[a, p+off] * T[a, co]  is a matmul on the
  TensorEngine with the per-tap gathered table T[(ci,kh,kw)][a, co] =
  lut[a, weight[co,ci,kh,kw]] (weight-side packing, precomputed on host) as
  the stationary operand and shifted windows of the one-hot plane as the
  moving operand, accumulating all 64*9*2 = 1152 matmuls per 512-column chunk
  into a persistent PSUM accumulator [128co x 3246pos].  Epilogue adds bias on
  the Scalar engine and DMAs out; host extracts the 56x56 valid columns.
"""

import os
import sys

sys.path.insert(0, "/opt/trn_rl_repo")

import numpy as np

B, CIN, H, W = 8, 64, 56, 56
COUT, K = 128, 3
HP, WP = H + 2, W + 2          # 58, 58 (pad=1)
NPIX = HP * WP                 # 3364
NOUT = (H - 1) * WP + W        # 3246: columns s = y*58+x, y,x in 0..55
N_CORES = 8
PSUM_CHUNK = 512
CHUNKS = [(c0, min(PSUM_CHUNK, NOUT - c0)) for c0 in range(0, NOUT, PSUM_CHUNK)]

_CACHE = {}


def _build_nc(n_ci=CIN):
    from contextlib import ExitStack

    import concourse.mybir as mybir
    import concourse.tile as tile
    from concourse import bacc

    nc = bacc.Bacc("TRN2", target_bir_lowering=False, debug=False)

    x = nc.dram_tensor("x", [CIN, NPIX], mybir.dt.int16, kind="ExternalInput").ap()
    t = nc.dram_tensor(
        "t", [CIN, 128, 18 * 128], mybir.dt.float16, kind="ExternalInput"
    ).ap()
    iota2 = nc.dram_tensor(
        "iota2", [128, 2], mybir.dt.float32, kind="ExternalInput"
    ).ap()
    bias = nc.dram_tensor(
        "bias", [128, 1], mybir.dt.float32, kind="ExternalInput"
    ).ap()
    y = nc.dram_tensor("y", [128, NOUT], mybir.dt.float32, kind="ExternalOutput").ap()

    fp16 = mybir.dt.float16
    fp32 = mybir.dt.float32
    i16 = mybir.dt.int16

    with tile.TileContext(nc) as tc, ExitStack() as ctx:
        const_pool = ctx.enter_context(tc.tile_pool(name="const", bufs=1))
        idx_pool = ctx.enter_context(tc.tile_pool(name="idx", bufs=3))
        t_pool = ctx.enter_context(tc.tile_pool(name="tt", bufs=3))
        oh_pool = ctx.enter_context(tc.tile_pool(name="oh", bufs=2))
        out_pool = ctx.enter_context(tc.tile_pool(name="outp", bufs=1))
        psum_pool = ctx.enter_context(tc.tile_pool(name="psum", bufs=1, space="PSUM"))

        iota_sb = const_pool.tile([128, 2], fp32)
        nc.sync.dma_start(iota_sb[:], iota2)
        bias_sb = const_pool.tile([128, 1], fp32)
        nc.sync.dma_start(bias_sb[:], bias)

        acc = psum_pool.tile([128, NOUT], fp32)

        for ci in range(n_ci):
            idx_rep = idx_pool.tile([128, NPIX], i16)
            nc.sync.dma_start(idx_rep[:], x[ci : ci + 1, :].to_broadcast((128, NPIX)))
            t_ci = t_pool.tile([128, 18 * 128], fp16)
            nc.sync.dma_start(t_ci[:], t[ci])

            oh_lo = oh_pool.tile([128, NPIX], fp16)
            oh_hi = oh_pool.tile([128, NPIX], fp16)
            nc.vector.tensor_scalar(
                oh_lo[:], idx_rep[:], iota_sb[:, 0:1], None, mybir.AluOpType.is_equal
            )
            nc.vector.tensor_scalar(
                oh_hi[:], idx_rep[:], iota_sb[:, 1:2], None, mybir.AluOpType.is_equal
            )

            for kh in range(K):
                for kw in range(K):
                    off = kh * WP + kw
                    for half, oh in ((0, oh_lo), (1, oh_hi)):
                        j = (kh * K + kw) * 2 + half
                        lhsT = t_ci[:, j * 128 : (j + 1) * 128]
                        first = ci == 0 and kh == 0 and kw == 0 and half == 0
                        last = (
                            ci == n_ci - 1 and kh == K - 1 and kw == K - 1 and half == 1
                        )
                        for c0, w in CHUNKS:
                            nc.tensor.matmul(
                                acc[:, c0 : c0 + w],
                                lhsT,
                                oh[:, off + c0 : off + c0 + w],
                                start=first,
                                stop=last,
                            )

        out_sb = out_pool.tile([128, NOUT], fp32)
        nc.scalar.activation(
            out_sb[:], acc[:], mybir.ActivationFunctionType.Identity, bias=bias_sb[:]
        )
        nc.sync.dma_start(y, out_sb[:])

    nc.compile()
    return nc


def _prep_host(input_np, weight_np, lut_np, bias_np):
    """Host-side packing: pad codes, gather per-tap tables from the LUT."""
    # Padded code planes, int16 (codes 0..255; pad contributes code 0 like ref)
    xpad = np.zeros((B, CIN, HP, WP), np.int16)
    xpad[:, :, 1 : 1 + H, 1 : 1 + W] = input_np
    xpad = xpad.reshape(B, CIN, NPIX)

    # T[ci, p, j= (kh*3+kw)*2+half, co] = lut[half*128+p, weight[co,ci,kh,kw]]
    wr = weight_np.astype(np.int64)                      # [co, ci, kh, kw]
    T = lut_np[:, wr]                                    # [a256, co, ci, kh, kw]
    T = T.transpose(2, 0, 3, 4, 1)                       # [ci, a256, kh, kw, co]
    T = T.reshape(CIN, 2, 128, K, K, COUT)               # [ci, half, p, kh, kw, co]
    T = T.transpose(0, 2, 3, 4, 1, 5)                    # [ci, p, kh, kw, half, co]
    T = np.ascontiguousarray(T.reshape(CIN, 128, 18 * 128)).astype(np.float16)

    iota2 = np.stack(
        [np.arange(128, dtype=np.float32), np.arange(128, 256, dtype=np.float32)], axis=1
    )
    bias_col = bias_np.reshape(128, 1).astype(np.float32)
    return xpad, T, iota2, bias_col


# column selector: valid output positions s = y*58 + x for y,x in 0..55
_SEL = (np.arange(H)[:, None] * WP + np.arange(W)[None, :]).ravel()


def _get_runner():
    global _CACHE
    if "nc" not in _CACHE:
        _CACHE["nc"] = _build_nc()
    return _CACHE["nc"]


def _run(input, weight, lut, bias, trace=False):
    input = np.asarray(input)
    weight = np.asarray(weight)
    lut = np.asarray(lut, dtype=np.float32)
    bias = np.asarray(bias, dtype=np.float32)

    xpad, T, iota2, bias_col = _prep_host(input, weight, lut, bias)

    nc = _get_runner()
    from concourse.bass_utils import run_bass_kernel_spmd

    in_maps = [
        {"x": xpad[b], "t": T, "iota2": iota2, "bias": bias_col} for b in range(B)
    ]
    res = run_bass_kernel_spmd(
        nc, in_maps, core_ids=list(range(N_CORES)), trace=trace
    )

    out = np.empty((B, COUT, H, W), np.float32)
    for b in range(B):
        yv = np.asarray(res.results[b]["y"])           # [128, NOUT]
        out[b] = yv[:, _SEL].reshape(COUT, H, W)
    return out, res


def kernel(input, weight, lut, bias):
    out, _ = _run(input, weight, lut, bias)
    return out


if __name__ == "__main__":
    # smoke test with random data
    rng = np.random.default_rng(0)
    inp = rng.integers(0, 256, (B, CIN, H, W), dtype=np.int32)
    wgt = rng.integers(0, 256, (COUT, CIN, K, K), dtype=np.int32)
    lut = rng.standard_normal((256, 256), dtype=np.float32)
    bias = rng.standard_normal((128,), dtype=np.float32)
    out = kernel(input=inp, weight=wgt, lut=lut, bias=bias)
    print(out.shape, out.dtype, out[0, 0, :2, :2])
